# revision 5
# baseline (speedup 1.0000x reference)
"""ESN+i (echo state network with input embedding) Trainium2 kernel.

Contract: kernel(**inputs) takes the FULL unsharded inputs
    x     (4, 2048) int       token ids
    emb   (32000, 2048) f32   embedding table
    W_rec (2048, 2048) f32    reservoir matrix
    W_out (32000, 2048) f32   output projection
    b_out (32000,) f32        output bias
and returns the FULL logits (4, 2048, 32000) f32, computed on 8
NeuronCores.

Strategy:
- The T=2048-step recurrence is sequential and tiny (batch 4); every core
  computes it redundantly in a transposed layout (hidden units on SBUF
  partitions, batch on the free axis) with W_rec resident in SBUF as the
  bf16 stationary operand: h'_{t} = 0.2 h_{t-1} + relu(0.8(W^T h + u)),
  with the 0.8 folded into W_rec and emb on the host.
- The embedding lookup uses the hardware gather DMA (dma_gather with
  transpose=True) straight into the transposed layout.
- The big output projection (8192x2048 @ 2048x32000) is tensor-parallel:
  each core owns a 4000-row vocab shard (padded to 4096) and projects the
  full hidden-state history, which stays resident in SBUF in bf16.
- Host prep: pre-transposed/packed weight layouts, int16 wrapped gather
  indices; host post: concatenate vocab shards and transpose to (B,T,V).
"""
import numpy as np
from contextlib import ExitStack

import concourse.bacc as bacc
import concourse.bass as bass
import concourse.mybir as mybir
import concourse.tile as tile
from concourse.bass_utils import run_bass_kernel_spmd

dt = mybir.dt
F32 = dt.float32
BF16 = dt.bfloat16

T, B, N, V = 2048, 4, 2048, 32000
VP = 4096          # padded per-core vocab shard (4000 -> 4096)
CHUNK = 128        # recurrence steps per chunk
NC = 8


RELU_DVE = False   # relu via DVE max (single-engine chain) vs ScalarE
UNROLL = 2         # recurrence steps per For_i body


def _build(repeat=1):
    KG = N // 128          # 16 hidden groups of 128
    TOKC = CHUNK * B       # tokens per chunk (512)
    NCHUNK = T // CHUNK

    nc = bacc.Bacc("TRN2", target_bir_lowering=False, debug=False,
                   num_devices=NC)

    embb = nc.dram_tensor("embb", [V, N], BF16, kind="ExternalInput")
    wrec = nc.dram_tensor("wrec", [128, KG * N], BF16, kind="ExternalInput")
    woutT = nc.dram_tensor("woutT", [128, KG, VP], BF16, kind="ExternalInput")
    biasw = nc.dram_tensor("biasw", [128, VP // 128], F32, kind="ExternalInput")
    xw = nc.dram_tensor("xw", [128, (T * B) // 16], dt.int16,
                        kind="ExternalInput")
    out = nc.dram_tensor("out", [VP, T * B], F32, kind="ExternalOutput")

    ACT_RELU = mybir.ActivationFunctionType.Relu
    MULT = mybir.AluOpType.mult
    ADD = mybir.AluOpType.add
    PE = mybir.EngineType.PE

    with tile.TileContext(nc) as tc, ExitStack() as ctx:
        wpool = ctx.enter_context(tc.tile_pool(name="w", bufs=1))
        stpool = ctx.enter_context(tc.tile_pool(name="state", bufs=1))
        upool = ctx.enter_context(tc.tile_pool(name="ut", bufs=2))
        hspool = ctx.enter_context(tc.tile_pool(name="hs", bufs=3))
        pspool = ctx.enter_context(tc.tile_pool(name="ps", bufs=2, space="PSUM"))
        vpool = ctx.enter_context(tc.tile_pool(name="vtmp", bufs=2))
        wopool = ctx.enter_context(tc.tile_pool(name="wo", bufs=3))
        pppool = ctx.enter_context(tc.tile_pool(name="pp", bufs=2, space="PSUM"))
        lgpool = ctx.enter_context(tc.tile_pool(name="lg", bufs=3))

        w_sb = wpool.tile([128, KG * N], BF16, tag="wrec")
        nc.sync.dma_start(w_sb[:], wrec[:, :])
        bias_sb = wpool.tile([128, VP // 128], F32, tag="bias")
        nc.sync.dma_start(bias_sb[:], biasw[:, :])
        idx_sb = wpool.tile([128, (T * B) // 16], dt.int16, tag="idx")
        nc.sync.dma_start(idx_sb[:], xw[:, :])

        hA = stpool.tile([128, KG * B], BF16, tag="hA")
        hB = stpool.tile([128, KG * B], BF16, tag="hB")

        def gather_chunk(c):
            ut = upool.tile([128, KG, TOKC], BF16, tag="ut")
            nc.gpsimd.dma_gather(
                out_ap=ut[:, :, :],
                in_ap=embb[:, :],
                idxs_ap=idx_sb[:, c * (TOKC // 16):(c + 1) * (TOKC // 16)],
                num_idxs=TOKC,
                num_idxs_reg=TOKC,
                elem_size=N,
                transpose=True,
            )
            return ut

        def emit_steps(ut_cur, hs_cur, off_expr):
            for par in range(UNROLL):
                src = hA if par % 2 == 0 else hB
                dst = hB if par % 2 == 0 else hA
                ps = pspool.tile([128, KG * B], F32, tag="ps")
                for m in range(KG):
                    for k in range(KG):
                        nc.tensor.matmul(
                            ps[:, m * B:(m + 1) * B],
                            w_sb[:, k * N + m * 128: k * N + (m + 1) * 128],
                            src[:, k * B:(k + 1) * B],
                            start=(k == 0),
                            stop=(k == KG - 1),
                        )
                u_ap = ut_cur[:, :, bass.ds(off_expr + par * B, B)]
                t1 = vpool.tile([128, KG * B], F32, tag="v1")
                nc.vector.tensor_add(t1[:], ps[:], u_ap)
                t2 = vpool.tile([128, KG * B], F32, tag="v2")
                if RELU_DVE:
                    nc.vector.tensor_scalar_max(t2[:], t1[:], 0.0)
                else:
                    nc.scalar.activation(t2[:], t1[:], ACT_RELU)
                nc.vector.scalar_tensor_tensor(dst[:], src[:], 0.2, t2[:],
                                               op0=MULT, op1=ADD)
                nc.scalar.copy(hs_cur[:, :, bass.ds(off_expr + par * B, B)],
                               dst[:])

        def recurrence_chunk(ut_cur, hs_cur):
            with tc.For_i(0, CHUNK // UNROLL, 1, hint_engines=(PE,)) as iv:
                emit_steps(ut_cur, hs_cur, iv * (UNROLL * B))

        def projection_chunk(c, hs_cur):
            for vt in range(VP // 128):
                wo = wopool.tile([128, KG, 128], BF16, tag="wo")
                nc.sync.dma_start(wo[:], woutT[:, :, vt * 128:(vt + 1) * 128])
                psp = pppool.tile([128, TOKC], F32, tag="pp")
                for k in range(KG):
                    nc.tensor.matmul(
                        psp[:],
                        wo[:, k, :],
                        hs_cur[:, k, :],
                        start=(k == 0),
                        stop=(k == KG - 1),
                    )
                lg = lgpool.tile([128, TOKC], F32, tag="lg")
                nc.vector.tensor_scalar_add(lg[:], psp[:],
                                            bias_sb[:, vt:vt + 1])
                nc.sync.dma_start(
                    out[vt * 128:(vt + 1) * 128, c * TOKC:(c + 1) * TOKC],
                    lg[:])

        def whole_pipeline():
            nc.vector.memset(hA[:], 0.0)
            ut_tiles = {0: gather_chunk(0)}
            for c in range(NCHUNK):
                if c + 1 < NCHUNK:
                    ut_tiles[c + 1] = gather_chunk(c + 1)
                hs_cur = hspool.tile([128, KG, TOKC], BF16, tag="hs")
                recurrence_chunk(ut_tiles[c], hs_cur)
                del ut_tiles[c]
                projection_chunk(c, hs_cur)

        if repeat == 1:
            whole_pipeline()
        else:
            # timing aid: run the whole pipeline `repeat` times in-NEFF
            with tc.For_i(0, repeat, 1):
                whole_pipeline()

    nc.compile()
    return nc


_NC_CACHE = None


def _get_nc():
    global _NC_CACHE
    if _NC_CACHE is None:
        _NC_CACHE = _build()
    return _NC_CACHE


def _prep_inputs(x, emb, W_rec, W_out, b_out):
    import ml_dtypes
    bf16 = ml_dtypes.bfloat16
    KG = N // 128

    embb = (0.8 * emb.astype(np.float32)).astype(bf16)
    wrec = (0.8 * W_rec.astype(np.float32)).astype(bf16) \
        .reshape(KG, 128, N).transpose(1, 0, 2).reshape(128, KG * N).copy()

    xf = np.ascontiguousarray(np.asarray(x).astype(np.int64).T) \
        .reshape(-1).astype(np.int16)             # token order 4t+b
    xw16 = xf.reshape(-1, 16).T.copy()
    xw = np.tile(xw16, (8, 1)).copy()             # [128, T*B/16]

    ins = []
    VS = V // NC
    for c in range(NC):
        Wo = np.zeros((VP, N), np.float32)
        Wo[:VS] = W_out[c * VS:(c + 1) * VS]
        bo = np.zeros((VP,), np.float32)
        bo[:VS] = b_out[c * VS:(c + 1) * VS]
        woutT = Wo.T.astype(bf16).reshape(KG, 128, VP).transpose(1, 0, 2).copy()
        biasw = bo.reshape(VP // 128, 128).T.copy()
        ins.append({"embb": embb, "wrec": wrec, "woutT": woutT,
                    "biasw": biasw, "xw": xw})
    return ins


def kernel(x, emb, W_rec, W_out, b_out):
    nc = _get_nc()
    ins = _prep_inputs(x, emb, W_rec, W_out, b_out)
    res = run_bass_kernel_spmd(nc, ins, core_ids=list(range(NC)))
    VS = V // NC
    full = np.empty((B, T, V), np.float32)
    for c in range(NC):
        o = res.results[c]["out"][:VS].reshape(VS, T, B)
        full[:, :, c * VS:(c + 1) * VS] = o.transpose(2, 1, 0)
    return full
